# revision 1
# baseline (speedup 1.0000x reference)
"""Trainium2 Bass kernel for nn_EnsembleAdaptor: batched per-member MLP.

Per ensemble member (32 total): y = relu(x @ w1.T + b1) @ w2.T + b2
with x (512, 1024), w1 (4096, 1024), b1 (4096), w2 (1024, 4096), b2 (1024).

Sharding: pure data parallel over members — 4 members per core across 8 cores.

Device algorithm per member (all matmuls fp16 with fp32 PSUM accumulation;
fp16 runs at the same 1 cycle/row PE rate as bf16 but with 10 mantissa bits):
  layer 1 computes hT (H on partitions): for each j-tile (32 of them),
    accumulate 8 k-tiles of   psum[j,s] += w1T_tile.T @ xT_tile   then
    ScalarE relu(psum + b1) -> hT sbuf tile (fp16).
  layer 2 computes yT (DOUT on partitions): for each o-tile (8), accumulate
    32 k-tiles of   psum[o,s] += w2T_tile.T @ hT_tile,  then ScalarE
    identity(psum + b2) -> f32 sbuf -> DMA out as yT (contiguous).

Host side packs weights/activations into the exact SBUF layouts the PE
needs (contraction dim on partitions), so every DMA is contiguous.
"""

import contextlib
import ctypes
import os
import sys
import types

import numpy as np
import ml_dtypes

import concourse.bass as bass
import concourse.tile as tile
from concourse import bacc, mybir
from concourse.bass_utils import run_bass_kernel_spmd


def _install_ntff_shim():
    """Provide antenv.axon_hooks + the ctypes NTFF profile hook when the
    image's antenv lacks them, so trace=True works under axon. Safe no-op
    on failure."""
    try:
        import antenv.axon_hooks  # noqa: F401
        return
    except ImportError:
        pass
    try:
        mod = types.ModuleType("antenv.axon_hooks")
        _state = {"hook": None}
        mod.set_axon_ntff_profile_hook = lambda h: _state.__setitem__("hook", h)
        mod.get_axon_ntff_profile_hook = lambda: _state["hook"]
        sys.modules["antenv.axon_hooks"] = mod
        import antenv
        antenv.axon_hooks = mod

        so_path = "/opt/axon/libaxon_pjrt.so"
        if not os.path.exists(so_path):
            return
        lib = ctypes.CDLL(so_path)
        if not hasattr(lib, "axon_start_nrt_profile"):
            return
        lib.axon_start_nrt_profile.argtypes = [
            ctypes.POINTER(ctypes.c_int64),
            ctypes.c_size_t,
        ]
        lib.axon_start_nrt_profile.restype = ctypes.c_int64
        lib.axon_stop_nrt_profile.argtypes = [ctypes.c_char_p]
        lib.axon_stop_nrt_profile.restype = ctypes.c_int64

        @contextlib.contextmanager
        def _hook(output_dir, device_ids):
            import jax
            jax.devices()
            if device_ids:
                ids = (ctypes.c_int64 * len(device_ids))(*device_ids)
                rc = lib.axon_start_nrt_profile(ids, len(device_ids))
            else:
                rc = lib.axon_start_nrt_profile(None, 0)
            if rc != 0:
                raise RuntimeError(f"axon_start_nrt_profile rc={rc}")
            try:
                yield
            finally:
                n = lib.axon_stop_nrt_profile(str(output_dir).encode())
                print(f"profile: {n} file(s) written to {output_dir}",
                      file=sys.stderr)

        mod.set_axon_ntff_profile_hook(_hook)
    except Exception:
        pass

B, S, DIN, H, DOUT = 32, 512, 1024, 4096, 1024
N_W1 = H * DIN
N_B1 = H
N_W2 = DOUT * H
N_B2 = DOUT

N_CORES = 8
M_PER = B // N_CORES  # members per core

DT = DIN // 128   # 8  k-tiles for layer 1
JT = H // 128     # 32 j-tiles (layer-1 outputs / layer-2 k-tiles)
OT = DOUT // 128  # 8  o-tiles for layer 2
SN = S            # 512 moving free dim

BF16 = mybir.dt.float16
F32 = mybir.dt.float32
NP_BF16 = np.float16

_cache = {}


def _build_nc():
    nc = bacc.Bacc("TRN2", target_bir_lowering=False, enable_partition_id=False)
    xp = nc.dram_tensor("xp", [M_PER, 128, DT * SN], BF16, kind="ExternalInput")
    w1p = nc.dram_tensor("w1p", [M_PER, JT, 128, DT * 128], BF16, kind="ExternalInput")
    w2p = nc.dram_tensor("w2p", [M_PER, OT, 128, JT * 128], BF16, kind="ExternalInput")
    b1p = nc.dram_tensor("b1p", [M_PER, 128, JT], F32, kind="ExternalInput")
    b2p = nc.dram_tensor("b2p", [M_PER, 128, OT], F32, kind="ExternalInput")
    ytp = nc.dram_tensor("ytp", [M_PER, OT, 128, SN], F32, kind="ExternalOutput")

    relu = mybir.ActivationFunctionType.Relu
    ident = mybir.ActivationFunctionType.Identity

    with tile.TileContext(nc) as tc:
        with (
            tc.tile_pool(name="xpool", bufs=2) as xpool,
            tc.tile_pool(name="w1pool", bufs=6) as w1pool,
            tc.tile_pool(name="w2pool", bufs=4) as w2pool,
            tc.tile_pool(name="bpool", bufs=2) as bpool,
            tc.tile_pool(name="hpool", bufs=2) as hpool,
            tc.tile_pool(name="ypool", bufs=4) as ypool,
            tc.tile_pool(name="ps1", bufs=4, space="PSUM") as ps1pool,
            tc.tile_pool(name="ps2", bufs=4, space="PSUM") as ps2pool,
        ):
            for m in range(M_PER):
                x_t = xpool.tile([128, DT * SN], BF16)
                w1_first = w1pool.tile([128, DT * 128], BF16, tag="w1_t")
                if m == 0:
                    # Critical head path: land the first matmul's operands
                    # before the bulk. Each dma_start costs ~0.6us of serial
                    # issue time on the Sync sequencer, but transfers run in
                    # parallel on separate HWDGE queues — so split just enough
                    # that the k=0..7 accumulation chain is never starved.
                    nc.sync.dma_start(x_t[:, 0:SN], xp[m, :, 0:SN])
                    nc.sync.dma_start(w1_first[:, 0:128], w1p[m, 0, :, 0:128])
                    nc.sync.dma_start(w1_first[:, 128:], w1p[m, 0, :, 128:])
                    nc.sync.dma_start(x_t[:, SN : 2 * SN], xp[m, :, SN : 2 * SN])
                    nc.sync.dma_start(
                        x_t[:, 2 * SN : 5 * SN], xp[m, :, 2 * SN : 5 * SN]
                    )
                    nc.sync.dma_start(x_t[:, 5 * SN :], xp[m, :, 5 * SN :])
                else:
                    nc.sync.dma_start(w1_first[:], w1p[m, 0])
                    nc.sync.dma_start(x_t[:], xp[m])
                b1_t = bpool.tile([128, JT], F32, tag="b1")
                nc.gpsimd.dma_start(b1_t[:], b1p[m])
                b2_t = bpool.tile([128, OT], F32, tag="b2")
                nc.gpsimd.dma_start(b2_t[:], b2p[m])

                h_t = hpool.tile([128, JT * SN], BF16)
                for jt in range(JT):
                    if jt == 0:
                        w1_t = w1_first
                    else:
                        w1_t = w1pool.tile([128, DT * 128], BF16, tag="w1_t")
                        nc.sync.dma_start(w1_t[:], w1p[m, jt])
                    ps = ps1pool.tile([128, SN], F32)
                    for k in range(DT):
                        nc.tensor.matmul(
                            ps[:],
                            w1_t[:, k * 128 : (k + 1) * 128],
                            x_t[:, k * SN : (k + 1) * SN],
                            start=(k == 0),
                            stop=(k == DT - 1),
                        )
                    nc.scalar.activation(
                        h_t[:, jt * SN : (jt + 1) * SN],
                        ps[:],
                        relu,
                        bias=b1_t[:, jt : jt + 1],
                    )

                for ot in range(OT):
                    w2_t = w2pool.tile([128, JT * 128], BF16)
                    nc.sync.dma_start(w2_t[:], w2p[m, ot])
                    if m == M_PER - 1 and ot == OT - 1:
                        # Last output tile: split into four 128-wide quarters
                        # so the earlier quarters' bias-add + store DMA overlap
                        # the later quarters' matmuls instead of serializing
                        # after the very last matmul.
                        for half in range(4):
                            lo = half * (SN // 4)
                            hi = lo + SN // 4
                            ps2 = ps2pool.tile([128, SN // 4], F32, tag="ps2")
                            for k in range(JT):
                                nc.tensor.matmul(
                                    ps2[:],
                                    w2_t[:, k * 128 : (k + 1) * 128],
                                    h_t[:, k * SN + lo : k * SN + hi],
                                    start=(k == 0),
                                    stop=(k == JT - 1),
                                )
                            y_t = ypool.tile([128, SN // 4], F32, tag="y_t")
                            nc.scalar.activation(
                                y_t[:], ps2[:], ident, bias=b2_t[:, ot : ot + 1]
                            )
                            nc.sync.dma_start(ytp[m, ot, :, lo:hi], y_t[:])
                        continue
                    ps2 = ps2pool.tile([128, SN], F32, tag="ps2")
                    for k in range(JT):
                        nc.tensor.matmul(
                            ps2[:],
                            w2_t[:, k * 128 : (k + 1) * 128],
                            h_t[:, k * SN : (k + 1) * SN],
                            start=(k == 0),
                            stop=(k == JT - 1),
                        )
                    y_t = ypool.tile([128, SN], F32, tag="y_t")
                    nc.scalar.activation(
                        y_t[:], ps2[:], ident, bias=b2_t[:, ot : ot + 1]
                    )
                    nc.sync.dma_start(ytp[m, ot], y_t[:])
    nc.compile()
    return nc


def _pack_core(x_flat, ensemble_weights, members):
    """Pack one core's members into the DMA-friendly device layouts."""
    n = len(members)
    xp = np.empty((n, 128, DT * SN), dtype=NP_BF16)
    w1p = np.empty((n, JT, 128, DT * 128), dtype=NP_BF16)
    w2p = np.empty((n, OT, 128, JT * 128), dtype=NP_BF16)
    b1p = np.empty((n, 128, JT), dtype=np.float32)
    b2p = np.empty((n, 128, OT), dtype=np.float32)
    for i, mem in enumerate(members):
        x = x_flat[mem].reshape(S, DIN)
        o = 0
        w1 = ensemble_weights[mem, o : o + N_W1].reshape(H, DIN); o += N_W1
        b1 = ensemble_weights[mem, o : o + N_B1]; o += N_B1
        w2 = ensemble_weights[mem, o : o + N_W2].reshape(DOUT, H); o += N_W2
        b2 = ensemble_weights[mem, o : o + N_B2]
        # xp[p, dt*S + s] = x[s, dt*128+p]
        xp[i] = (
            x.reshape(S, DT, 128).transpose(2, 1, 0).reshape(128, DT * SN)
        ).astype(NP_BF16)
        # w1p[jt, p, dt*128+jj] = w1[jt*128+jj, dt*128+p]
        w1p[i] = (
            w1.reshape(JT, 128, DT, 128)
            .transpose(0, 3, 2, 1)
            .reshape(JT, 128, DT * 128)
        ).astype(NP_BF16)
        # w2p[ot, p, jt*128+oo] = w2[ot*128+oo, jt*128+p]
        w2p[i] = (
            w2.reshape(OT, 128, JT, 128)
            .transpose(0, 3, 2, 1)
            .reshape(OT, 128, JT * 128)
        ).astype(NP_BF16)
        b1p[i] = b1.reshape(JT, 128).T.astype(np.float32)
        b2p[i] = b2.reshape(OT, 128).T.astype(np.float32)
    return {"xp": xp, "w1p": w1p, "w2p": w2p, "b1p": b1p, "b2p": b2p}


def kernel(x_flat: np.ndarray, ensemble_weights: np.ndarray) -> np.ndarray:
    x_flat = np.asarray(x_flat, dtype=np.float32)
    ensemble_weights = np.asarray(ensemble_weights, dtype=np.float32)

    if "nc" not in _cache:
        _cache["nc"] = _build_nc()
    nc = _cache["nc"]

    in_maps = [
        _pack_core(x_flat, ensemble_weights,
                   list(range(c * M_PER, (c + 1) * M_PER)))
        for c in range(N_CORES)
    ]

    trace = bool(int(os.environ.get("KERNEL_TRACE", "0")))
    if trace:
        _install_ntff_shim()
    res = run_bass_kernel_spmd(nc, in_maps, core_ids=list(range(N_CORES)),
                               trace=trace)
    if trace:
        _cache["exec_time_ns"] = res.exec_time_ns

    out = np.empty((B, S * DOUT), dtype=np.float32)
    for c in range(N_CORES):
        ytp = res.results[c]["ytp"]  # (M_PER, OT, 128, SN)
        for i in range(M_PER):
            mem = c * M_PER + i
            # y[s, ot*128+p] = ytp[i, ot, p, s]
            out[mem] = (
                ytp[i].transpose(2, 0, 1).reshape(S * DOUT).astype(np.float32)
            )
    return out

